# revision 2
# baseline (speedup 1.0000x reference)
"""Cost-volume kernel for Trainium2 (Bass/Tile), SPMD over 8 NeuronCores.

out[b,c,d,h,w] = left[b,c,h,w] * right[b,c,h,w-d]  (0 where w < d), clipped.

Sharding: channels C=32 split 4-per-core (identical SPMD program, the cores
differ only in the input data they receive). Each core computes its
[4, 64, 160, 320] output slab; the host concatenates along C.

Per-core layout: rows (c,h) on the 128 SBUF partitions.
  - tiles 0..3: channel c, h in [0,128)          -> [128, W]
  - tile  4   : all 4 channels, h in [128,160)   -> [4*32, W] packed
The disparity shift is along W only, so rows are independent and the shifted
product for disparity d is  out_tile[:, w'] = l[:, d+w'] * r[:, w']  for
w' in [0, W-d).  Eight disparities are batched into ONE DVE tensor_tensor op
via a 3-D access pattern:
    out[p, j, w'] = l[p, d0+j+w'] * r[p, w']   (j = 0..7, r broadcast over j)
which cuts the op count 8x at ~1% redundant compute.

Only the valid region w >= d is DMA'd to DRAM; the masked w < d region is
zero-filled on the host.
"""

import numpy as np

import concourse.bass as bass
import concourse.tile as tile
from concourse import bacc, mybir
from concourse.bass_utils import run_bass_kernel_spmd

B, C, H, W = 1, 32, 160, 320
D = 64
N_CORES = 8
C_LOC = C // N_CORES          # 4 channels per core
JBLK = 8                      # disparities per DVE op
LPAD = W + JBLK               # left tiles padded so the window AP stays in bounds
H_MAIN = 128                  # h rows on partitions for the per-channel main tiles
H_TAIL = H - H_MAIN           # 32
N_TILES = C_LOC + 1           # 4 main + 1 packed tail

_cache = {}


def _build_program():
    nc = bacc.Bacc(
        "TRN2",
        target_bir_lowering=False,
        debug=False,
        enable_asserts=True,
        num_devices=N_CORES,
    )
    left_d = nc.dram_tensor(
        "left", [C_LOC, H, W], mybir.dt.float32, kind="ExternalInput"
    ).ap()
    right_d = nc.dram_tensor(
        "right", [C_LOC, H, W], mybir.dt.float32, kind="ExternalInput"
    ).ap()
    out_d = nc.dram_tensor(
        "out", [C_LOC, D, H, W], mybir.dt.float32, kind="ExternalOutput"
    ).ap()

    lts = [
        nc.alloc_sbuf_tensor(f"lt{t}", [128, LPAD], mybir.dt.float32).ap()
        for t in range(N_TILES)
    ]
    rts = [
        nc.alloc_sbuf_tensor(f"rt{t}", [128, W], mybir.dt.float32).ap()
        for t in range(N_TILES)
    ]

    with tile.TileContext(nc) as tc:
        with tc.tile_pool(name="outp", bufs=4) as outp:
            # Zero the pad columns of the left tiles: the window AP reads up to
            # JBLK-1 columns past W; those products are never stored but must
            # not be uninitialized.
            for t in range(N_TILES):
                nc.vector.memset(lts[t][:, W:LPAD], 0.0)

            # Loads. Main tiles: one channel, h in [0,128).
            for t in range(C_LOC):
                nc.sync.dma_start(out=lts[t][:, 0:W], in_=left_d[t, 0:H_MAIN, :])
                nc.sync.dma_start(out=rts[t][:, 0:W], in_=right_d[t, 0:H_MAIN, :])
            # Packed tail tile: 4 channels x 32 rows.
            for c in range(C_LOC):
                p0 = c * H_TAIL
                nc.sync.dma_start(
                    out=lts[C_LOC][p0 : p0 + H_TAIL, 0:W],
                    in_=left_d[c, H_MAIN:H, :],
                )
                nc.sync.dma_start(
                    out=rts[C_LOC][p0 : p0 + H_TAIL, 0:W],
                    in_=right_d[c, H_MAIN:H, :],
                )

            for d0 in range(0, D, JBLK):
                n = W - d0  # widest disparity in the block
                for t in range(N_TILES):
                    blk = outp.tile(
                        [128, JBLK, W], mybir.dt.float32, name=f"blk_{d0}_{t}", tag="blk"
                    )
                    l_base = lts[t][:, :]
                    pitch = l_base.ap[0][0]
                    l_win = bass.AP(
                        l_base.tensor,
                        l_base.offset + d0,
                        [[pitch, 128], [1, JBLK], [1, n]],
                    )
                    r_bc = rts[t][:, 0:n].unsqueeze(1).broadcast_to([128, JBLK, n])
                    nc.vector.tensor_mul(blk[:, :, 0:n], l_win, r_bc)

                    for j in range(JBLK):
                        d = d0 + j
                        wv = W - d  # valid width for this disparity
                        if t < C_LOC:
                            nc.sync.dma_start(
                                out=out_d[t, d, 0:H_MAIN, d:W],
                                in_=blk[:, j, 0:wv],
                            )
                        else:
                            for c in range(C_LOC):
                                p0 = c * H_TAIL
                                nc.sync.dma_start(
                                    out=out_d[c, d, H_MAIN:H, d:W],
                                    in_=blk[p0 : p0 + H_TAIL, j, 0:wv],
                                )

    nc.compile()
    return nc


def kernel(**inputs):
    left = np.ascontiguousarray(np.asarray(inputs["left"], dtype=np.float32))
    right = np.ascontiguousarray(np.asarray(inputs["right"], dtype=np.float32))
    nd = int(np.asarray(inputs["num_disparities"]))
    assert left.shape == (B, C, H, W) and right.shape == (B, C, H, W)
    assert nd == D, f"kernel hardcodes num_disparities={D}, got {nd}"

    if "nc" not in _cache:
        _cache["nc"] = _build_program()
    nc = _cache["nc"]

    in_maps = [
        {
            "left": np.ascontiguousarray(left[0, i * C_LOC : (i + 1) * C_LOC]),
            "right": np.ascontiguousarray(right[0, i * C_LOC : (i + 1) * C_LOC]),
        }
        for i in range(N_CORES)
    ]
    res = run_bass_kernel_spmd(nc, in_maps, list(range(N_CORES)))
    _cache["last_results"] = res

    full = np.concatenate(
        [np.asarray(r["out"]) for r in res.results], axis=0
    )  # (32, 64, 160, 320)
    # Zero the masked region w < d (never written on device).
    for d in range(1, D):
        full[:, d, :, :d] = 0.0
    np.clip(full, -1000.0, 1000.0, out=full)
    return full[None].astype(np.float32)  # (1, 32, 64, 160, 320)


# revision 7
# speedup vs baseline: 2.3503x; 2.3503x over previous
"""Cost-volume kernel for Trainium2 (Bass/Tile), SPMD over 8 NeuronCores.

out[b,c,d,h,w] = left[b,c,h,w] * right[b,c,h,w-d]  (0 where w < d), clipped.

Sharding: channels C=32 split 4-per-core (identical SPMD program, cores differ
only in input data). Each core computes its [4, 64, 160, 320] slab; the host
concatenates along C.

Per-core layout: rows (c,h) on the 128 SBUF partitions.
  - tiles 0..3: channel c, h in [0,128)          -> [128, W]
  - tile  4   : all 4 channels, h in [128,160)   -> [4*32, W] packed
The disparity shift is along W only, so rows are independent:
  out_tile[:, w'] = l[:, d+w'] * r[:, w']   for w' in [0, W-d).
Eight disparities are batched into ONE DVE tensor_tensor op via a 3-D access
pattern (r broadcast over j with a stride-0 dim, l as an overlapping window):
  blk[p, j, w'] = l[p, d0+j+w'] * r[p, w']       (j = 0..7)

Output DMAs are batched 8-disparities-at-a-time with a diagonal DRAM access
pattern: for the [C,D,H,W] d-major layout, element (p=h, j, w') lands at
  c*D*H*W + (d0+j)*H*W + h*W + (d0+j) + w'  =  base + h*W + j*(H*W+1) + w'
i.e. DRAM AP [[W,128],[H*W+1,8],[1,N]] with N = W-d0 -- one DMA per
(tile, block) instead of one per (tile, disparity).  Rows where w' exceeds
the valid width for j spill into the *masked* (w<d) region of the next h row
and multiply l's zeroed pad columns, so they write harmless zeros. The only
unsafe corner is the packed-tail tile's last block (would run past the end of
the tensor), which is emitted as 8 exact per-disparity DMAs instead.

The remaining unwritten masked cells are zero-filled on the host.
"""

import os

import numpy as np

os.environ.setdefault("NEURON_RT_RESET_CORES", "1")

import concourse.bass as bass
import concourse.tile as tile
from concourse import bacc, mybir
from concourse.bass_utils import run_bass_kernel_spmd

B, C, H, W = 1, 32, 160, 320
D = 64
N_CORES = 8
C_LOC = C // N_CORES          # 4 channels per core
JBLK = 8                      # disparities per DVE op / per output DMA
LPAD = W + JBLK               # left tiles padded so the window AP stays in bounds
H_MAIN = 128                  # h rows on partitions for the per-channel main tiles
H_TAIL = H - H_MAIN           # 32
N_TILES = C_LOC + 1           # 4 main + 1 packed tail

OUT_C_STRIDE = D * H * W      # element strides of the [C_LOC, D, H, W] output
OUT_D_STRIDE = H * W
DIAG = OUT_D_STRIDE + 1       # +1: each next disparity starts one column later

_cache = {}


def _build_program():
    nc = bacc.Bacc(
        "TRN2",
        target_bir_lowering=False,
        debug=False,
        enable_asserts=True,
        num_devices=N_CORES,
    )
    left_d = nc.dram_tensor(
        "left", [C_LOC, H, W], mybir.dt.float32, kind="ExternalInput"
    ).ap()
    right_d = nc.dram_tensor(
        "right", [C_LOC, H, W], mybir.dt.float32, kind="ExternalInput"
    ).ap()
    out_d = nc.dram_tensor(
        "out", [C_LOC, D, H, W], mybir.dt.float32, kind="ExternalOutput"
    ).ap()
    out_t = out_d.tensor

    lts = [
        nc.alloc_sbuf_tensor(f"lt{t}", [128, LPAD], mybir.dt.float32).ap()
        for t in range(N_TILES)
    ]
    rts = [
        nc.alloc_sbuf_tensor(f"rt{t}", [128, W], mybir.dt.float32).ap()
        for t in range(N_TILES)
    ]

    with tile.TileContext(nc) as tc:
        with tc.tile_pool(name="outp", bufs=6) as outp:
            # Zero the pad columns of the left tiles: the window AP reads up
            # to JBLK-1 columns past W; those products spill into masked
            # output cells, so they must be zero (l_pad * r = 0).
            for t in range(N_TILES):
                nc.vector.memset(lts[t][:, W:LPAD], 0.0)

            # Loads. Main tiles: one channel, h in [0,128).
            for t in range(C_LOC):
                nc.sync.dma_start(out=lts[t][:, 0:W], in_=left_d[t, 0:H_MAIN, :])
                nc.sync.dma_start(out=rts[t][:, 0:W], in_=right_d[t, 0:H_MAIN, :])
            # Packed tail tile: 4 channels x 32 rows.
            for c in range(C_LOC):
                p0 = c * H_TAIL
                nc.sync.dma_start(
                    out=lts[C_LOC][p0 : p0 + H_TAIL, 0:W],
                    in_=left_d[c, H_MAIN:H, :],
                )
                nc.sync.dma_start(
                    out=rts[C_LOC][p0 : p0 + H_TAIL, 0:W],
                    in_=right_d[c, H_MAIN:H, :],
                )

            for d0 in range(0, D, JBLK):
                n = W - d0  # width of the j=0 (widest) disparity in the block
                last_blk = d0 == D - JBLK
                for t in range(N_TILES):
                    blk = outp.tile(
                        [128, JBLK, W], mybir.dt.float32, name=f"blk_{d0}_{t}", tag="blk"
                    )
                    bb = blk[:, :, :]
                    bpitch, bjs = bb.ap[0][0], bb.ap[1][0]

                    l_base = lts[t][:, :]
                    lpitch = l_base.ap[0][0]
                    l_win = bass.AP(
                        l_base.tensor,
                        l_base.offset + d0,
                        [[lpitch, 128], [1, JBLK], [1, n]],
                    )
                    r_bc = rts[t][:, 0:n].unsqueeze(1).broadcast_to([128, JBLK, n])
                    nc.vector.tensor_mul(blk[:, :, 0:n], l_win, r_bc)

                    sb_blk = bass.AP(
                        bb.tensor, bb.offset, [[bpitch, 128], [bjs, JBLK], [1, n]]
                    )
                    if t < C_LOC:
                        # Diagonal-batched store, h in [0,128) of channel t.
                        dram = bass.AP(
                            out_t,
                            t * OUT_C_STRIDE + d0 * DIAG,
                            [[W, 128], [DIAG, JBLK], [1, n]],
                        )
                        nc.sync.dma_start(out=dram, in_=sb_blk)
                    elif not last_blk:
                        # Packed tail: DMA APs allow at most 3 dims, so one
                        # diagonal store per channel (h, j, w').
                        for c in range(C_LOC):
                            bb_c = blk[c * H_TAIL : (c + 1) * H_TAIL, :, :]
                            sb_c = bass.AP(
                                bb_c.tensor,
                                bb_c.offset,
                                [[bpitch, H_TAIL], [bjs, JBLK], [1, n]],
                            )
                            dram = bass.AP(
                                out_t,
                                c * OUT_C_STRIDE + H_MAIN * W + d0 * DIAG,
                                [[W, H_TAIL], [DIAG, JBLK], [1, n]],
                            )
                            nc.sync.dma_start(out=dram, in_=sb_c)
                    else:
                        # Tail tile, last block: the diagonal spill of the very
                        # last row would run past the tensor end -> exact
                        # per-disparity stores.
                        for j in range(JBLK):
                            d = d0 + j
                            wv = W - d
                            dram = bass.AP(
                                out_t,
                                H_MAIN * W + d * OUT_D_STRIDE + d,
                                [[OUT_C_STRIDE, C_LOC], [W, H_TAIL], [1, wv]],
                            )
                            nc.sync.dma_start(out=dram, in_=blk[:, j, 0:wv])

    nc.compile()
    return nc


def kernel(**inputs):
    left = np.ascontiguousarray(np.asarray(inputs["left"], dtype=np.float32))
    right = np.ascontiguousarray(np.asarray(inputs["right"], dtype=np.float32))
    nd = int(np.asarray(inputs["num_disparities"]))
    assert left.shape == (B, C, H, W) and right.shape == (B, C, H, W)
    assert nd == D, f"kernel hardcodes num_disparities={D}, got {nd}"

    if "nc" not in _cache:
        _cache["nc"] = _build_program()
    nc = _cache["nc"]

    in_maps = [
        {
            "left": np.ascontiguousarray(left[0, i * C_LOC : (i + 1) * C_LOC]),
            "right": np.ascontiguousarray(right[0, i * C_LOC : (i + 1) * C_LOC]),
        }
        for i in range(N_CORES)
    ]
    res = run_bass_kernel_spmd(nc, in_maps, list(range(N_CORES)))
    _cache["last_results"] = res

    full = np.concatenate(
        [np.asarray(r["out"]) for r in res.results], axis=0
    )  # (32, 64, 160, 320)
    # Zero the masked region w < d (only partially written on device).
    for d in range(1, D):
        full[:, d, :, :d] = 0.0
    np.clip(full, -1000.0, 1000.0, out=full)
    return full[None]  # (1, 32, 64, 160, 320) float32
